# revision 7
# baseline (speedup 1.0000x reference)
"""AGNNConv on 8 Trainium2 NeuronCores — dense matmul formulation.

The per-edge attention weight exp(beta * cos(src, dst)) depends only on the
(src, dst) node pair, so the whole message passing collapses to dense algebra:

    G = norm^T norm                  (Gram matrix of L2-normalized features)
    H = C  *  exp(beta * G)          (C = dense dst-by-src edge-count matrix)
    out = (H @ [feat | 1]) ;  out = num / denom  rowwise

The count matrix C is built on the host from the edge list; every random
gather/scatter of the reference becomes streaming dense work on device.

Sharding: destination nodes (columns of H^T) are split across the 8 cores;
each core computes its 1280 output rows end-to-end.  No collectives needed.
"""

import sys
import types

import numpy as np

try:
    from concourse import bacc, mybir, tile, masks
    from concourse.bass_utils import run_bass_kernel_spmd
except ImportError:  # harness container may not have the repo on sys.path
    for _p in ("/opt/trn_rl_repo", "/root/.axon_site/_ro/trn_rl_repo"):
        if _p not in sys.path:
            sys.path.append(_p)
    from concourse import bacc, mybir, tile, masks
    from concourse.bass_utils import run_bass_kernel_spmd

F32 = mybir.dt.float32
BF16 = mybir.dt.bfloat16
U8 = mybir.dt.uint8
AF = mybir.ActivationFunctionType
ALU = mybir.AluOpType

D = 128  # feature dim


def make_cfg(n_nodes=10000, npad=10240, ncores=8, nchunk=256, gm=4):
    c = types.SimpleNamespace()
    c.n_nodes = n_nodes
    c.npad = npad              # padded node count (multiple of 128*ncores)
    c.ncores = ncores
    c.npc = npad // ncores     # dst columns per core
    c.nchunk = nchunk          # dst columns per j-chunk
    c.nj = c.npc // nchunk     # j-chunks per core
    c.ns = nchunk // 128       # psum_o accumulators per j-chunk
    c.mch = npad // 128        # source-node chunks (contraction dim)
    c.gm = gm                  # m-chunks per exp group
    c.ng = c.mch // gm         # exp groups
    c.tt = c.npc // 128        # output row-tiles per core
    assert c.npc % nchunk == 0 and nchunk % 128 == 0 and c.mch % gm == 0
    return c


def build(cfg):
    """Build the per-core SPMD graph (identical on all cores; data differs)."""
    nc = bacc.Bacc(
        "TRN2", target_bir_lowering=False, debug=False, num_devices=cfg.ncores
    )
    featd = nc.dram_tensor("feat_pad", [cfg.npad, D], F32, kind="ExternalInput")
    fmyd = nc.dram_tensor("feat_my", [cfg.npc, D], F32, kind="ExternalInput")
    ctd = nc.dram_tensor(
        "ct", [cfg.nj, 128, cfg.mch * cfg.nchunk], U8, kind="ExternalInput"
    )
    betad = nc.dram_tensor("beta128", [128, 1], F32, kind="ExternalInput")
    outd = nc.dram_tensor("out", [128, cfg.tt, D], F32, kind="ExternalOutput")

    nck = cfg.nchunk
    D1 = D + 1

    with tile.TileContext(nc) as tc:
        with (
            tc.tile_pool(name="const", bufs=1) as constp,
            tc.tile_pool(name="big", bufs=1) as bigp,
            tc.tile_pool(name="nrm", bufs=6) as nrmp,
            tc.tile_pool(name="cb", bufs=2) as cbp,
            tc.tile_pool(name="eg", bufs=3) as egp,
            tc.tile_pool(name="ht", bufs=4) as htp,
            tc.tile_pool(name="pt", bufs=2, space="PSUM") as ptp,
            tc.tile_pool(name="pg", bufs=2, space="PSUM") as pgp,
            tc.tile_pool(name="po", bufs=cfg.ns, space="PSUM") as pop,
        ):
            ident = constp.tile([128, 128], BF16)
            masks.make_identity(nc, ident[:])
            beta_sb = constp.tile([128, 1], F32)
            nc.sync.dma_start(beta_sb[:], betad[:])

            nchnk_all = cfg.mch + cfg.tt  # feat_pad chunks + feat_my chunks
            ssbuf = constp.tile([128, nchnk_all], F32)
            dflr = constp.tile([128, nchnk_all], F32)
            rnbuf = constp.tile([128, nchnk_all], F32)

            fbig = bigp.tile([128, cfg.mch, D], F32)
            fmy = bigp.tile([128, cfg.tt, D], F32)
            normT = bigp.tile([128, cfg.npad], BF16)
            normTmy = bigp.tile([128, cfg.npc], BF16)
            featq = bigp.tile([128, cfg.mch * D1], BF16)
            outacc = bigp.tile([128, cfg.nj * cfg.ns, D1], F32)
            final = bigp.tile([128, cfg.npc], F32)

            nc.vector.memset(featq[:], 1.0)  # bias column = 1; rest overwritten

            # ---- load features: [npad, D] -> [128, mch, D] (row i*128+p -> (p, i)) ----
            nsplit = 4
            step = cfg.mch // nsplit if cfg.mch >= nsplit else cfg.mch
            for a in range(0, cfg.mch, step):
                b = min(a + step, cfg.mch)
                nc.sync.dma_start(
                    fbig[:, a:b, :],
                    featd[:].rearrange("(m p) d -> p m d", p=128)[:, a:b, :],
                )
            nc.sync.dma_start(
                fmy[:], fmyd[:].rearrange("(m p) d -> p m d", p=128)
            )

            # ---- sum of squares per node ----
            def sumsq(view, col):
                sqscr = nrmp.tile([128, D], F32, tag="sq", name="sqscr")
                nc.vector.tensor_mul(sqscr[:], view, view)
                nc.vector.tensor_reduce(
                    ssbuf[:, col : col + 1], sqscr[:],
                    axis=mybir.AxisListType.X, op=ALU.add,
                )

            for i in range(cfg.mch):
                sumsq(fbig[:, i, :], i)
            for t in range(cfg.tt):
                sumsq(fmy[:, t, :], cfg.mch + t)
            # rn = 1 / max(sqrt(ss), 1e-12)
            nc.scalar.activation(dflr[:], ssbuf[:], AF.Sqrt)
            nc.vector.tensor_scalar(
                out=dflr[:], in0=dflr[:], scalar1=1e-12, scalar2=None, op0=ALU.max
            )
            nc.vector.reciprocal(rnbuf[:], dflr[:])

            # ---- normalize + transpose into normT / normTmy ----
            def norm_transpose(src_view, col, dst, dcol):
                nrm = nrmp.tile([128, D], BF16, tag="nrm")
                nc.vector.tensor_scalar(
                    out=nrm[:], in0=src_view, scalar1=rnbuf[:, col : col + 1],
                    scalar2=None, op0=ALU.mult,
                )
                pt = ptp.tile([128, 128], BF16, tag="pt")
                nc.tensor.transpose(pt[:], nrm[:], ident[:])
                nc.scalar.copy(dst[:, dcol : dcol + 128], pt[:])

            for i in range(cfg.mch):
                norm_transpose(fbig[:, i, :], i, normT, i * 128)
                nc.gpsimd.tensor_copy(featq[:, i * D1 : i * D1 + D], fbig[:, i, :])
            for t in range(cfg.tt):
                norm_transpose(fmy[:, t, :], cfg.mch + t, normTmy, t * 128)

            # ---- main loop over dst-column chunks ----
            for j in range(cfg.nj):
                cb = cbp.tile([128, cfg.mch * nck], U8, tag="cb")
                nc.sync.dma_start(cb[:], ctd[j, :, :])
                po = [
                    pop.tile([128, D1], F32, tag="po", name=f"po{s}")
                    for s in range(cfg.ns)
                ]
                for g in range(cfg.ng):
                    pg = pgp.tile([128, cfg.gm * nck], F32, tag="pg")
                    for k in range(cfg.gm):
                        i = g * cfg.gm + k
                        nc.tensor.matmul(
                            pg[:, k * nck : (k + 1) * nck],
                            normT[:, i * 128 : (i + 1) * 128],
                            normTmy[:, j * nck : (j + 1) * nck],
                            start=True, stop=True,
                        )
                    eg = egp.tile([128, cfg.gm * nck], BF16, tag="eg")
                    nc.scalar.activation(
                        eg[:], pg[:], AF.Exp, scale=beta_sb[:, 0:1]
                    )
                    ht = htp.tile([128, cfg.gm * nck], BF16, tag="ht")
                    nc.vector.tensor_tensor(
                        ht[:], eg[:],
                        cb[:, g * cfg.gm * nck : (g + 1) * cfg.gm * nck],
                        op=ALU.mult,
                    )
                    for k in range(cfg.gm):
                        i = g * cfg.gm + k
                        for s in range(cfg.ns):
                            nc.tensor.matmul(
                                po[s][:],
                                ht[:, k * nck + s * 128 : k * nck + (s + 1) * 128],
                                featq[:, i * D1 : (i + 1) * D1],
                                start=(g == 0 and k == 0),
                                stop=(g == cfg.ng - 1 and k == cfg.gm - 1),
                                skip_group_check=True,
                            )
                for s in range(cfg.ns):
                    t = j * cfg.ns + s
                    nc.vector.tensor_copy(outacc[:, t, :], po[s][:])

            # ---- out = num / max(denom, tiny) ----
            dmax = constp.tile([128, cfg.tt], F32)
            rden = constp.tile([128, cfg.tt], F32)
            nc.vector.tensor_scalar(
                out=dmax[:], in0=outacc[:, :, D : D + 1], scalar1=1e-30,
                scalar2=None, op0=ALU.max,
            )
            nc.vector.reciprocal(rden[:], dmax[:])
            for t in range(cfg.tt):
                nc.vector.tensor_scalar(
                    out=final[:, t * D : (t + 1) * D], in0=outacc[:, t, 0:D],
                    scalar1=rden[:, t : t + 1], scalar2=None, op0=ALU.mult,
                )
            nc.sync.dma_start(outd[:], final[:].rearrange("p (t d) -> p t d", d=D))

    nc.compile()
    return nc


def prepare_inputs(feat, src, dst, beta, cfg):
    feat = np.ascontiguousarray(np.asarray(feat), dtype=np.float32)
    src = np.asarray(src).astype(np.int64)
    dst = np.asarray(dst).astype(np.int64)
    beta = np.asarray(beta, dtype=np.float32).reshape(-1)

    featp = np.zeros((cfg.npad, D), np.float32)
    featp[: cfg.n_nodes] = feat
    beta128 = np.full((128, 1), beta[0], np.float32)

    in_maps = []
    for c in range(cfg.ncores):
        lo = c * cfg.npc
        m = (dst >= lo) & (dst < lo + cfg.npc)
        s_c = src[m]
        d_c = dst[m] - lo
        cnt = np.bincount(
            s_c * cfg.npc + d_c, minlength=cfg.npad * cfg.npc
        ).reshape(cfg.npad, cfg.npc)
        assert cnt.max() <= 255, "uint8 range exceeded"
        ct = np.empty(
            (cfg.nj, 128, cfg.mch * cfg.nchunk), dtype=np.uint8
        )
        for j in range(cfg.nj):
            blk = cnt[:, j * cfg.nchunk : (j + 1) * cfg.nchunk]
            blk = blk.reshape(cfg.mch, 128, cfg.nchunk).transpose(1, 0, 2)
            ct[j] = blk.reshape(128, cfg.mch * cfg.nchunk).astype(np.uint8)
        in_maps.append(
            {
                "feat_pad": featp,
                "feat_my": np.ascontiguousarray(featp[lo : lo + cfg.npc]),
                "ct": ct,
                "beta128": beta128,
            }
        )
    return in_maps


def postprocess(results, cfg):
    parts = []
    for c in range(cfg.ncores):
        o = np.asarray(results[c]["out"], np.float32)  # [128, tt, D]
        parts.append(o.transpose(1, 0, 2).reshape(cfg.npc, D))
    return np.concatenate(parts, axis=0)[: cfg.n_nodes]


_CACHE = {}


def _get_nc(cfg):
    key = (cfg.npad, cfg.ncores, cfg.nchunk, cfg.gm)
    if key not in _CACHE:
        _CACHE[key] = build(cfg)
    return _CACHE[key]


def kernel(feat, src, dst, beta):
    cfg = make_cfg()
    nc = _get_nc(cfg)
    in_maps = prepare_inputs(feat, src, dst, beta, cfg)
    res = run_bass_kernel_spmd(nc, in_maps, core_ids=list(range(cfg.ncores)))
    return postprocess(res.results, cfg)
